# revision 6
# baseline (speedup 1.0000x reference)
"""Causal self-attention (B=2, T=2048, C=1024, H=16) on 8 trn2 NeuronCores.

Sharding: core c handles batch b=c//4 and head group g=c%4 (4 heads each).
Data parallel on B, tensor parallel on H; W_attn/W_proj sliced per head
group; host sums the 4 tensor-parallel partial projection outputs per batch.

v3 layout notes (v2 spent 24us on the DMA lead-in -- each dma_start costs
~0.6us serially on the sync sequencer at ~116 descriptors/call, and
b_qk-as-[128,1] alone was 512 4-byte descriptors -- and ran the first ~35us
of matmuls at mid p-state):
  - all big inputs are HOST-PACKED so each SBUF tile is a contiguous DRAM
    block of 128 rows x (multi-KB); every fetch is 16 dma_starts of
    [8 partitions, whole-row] slices -> 8 descriptors/call (~100ns issue),
    one call per DMA queue. First qkT matmul inputs land ~7us in.
  - b_qk comes as [1, 512] (one descriptor) and is transposed to 4x[128,1]
    on the PE (permutation-matmul) during the lead-in.
  - ~20 warm-up matmuls on a dummy tile run while DMA streams, so the HAM
    p-state ramp (full speed only after 3us of continuous PE execution)
    completes before real work starts.
  - qkT runs only column-chunks 0/1 (queries 0:1024) up front; chunks 2/3
    are issued as fillers inside the pair-0 attention stream. q/k tiles are
    split per 1024-column zone (qkA/qkB) so pair-0 readers never serialize
    against the in-flight zone-B writers.
  - attention per (pair, head): S^T per 128-key block from the exact causal
    column into [128,1024] psum; ACT exp (scale=1/8) -> bf16 pT; mixed
    diagonal block masked multiplicatively on DVE; PV issued one block late
    with v tiles whose per-head 128 columns = [64 ones | 64 v], so psum
    rows 0..63 replicate the softmax denominator (reciprocal_approx_fast
    directly on [64,N], no partition broadcast).
  - filler queue (v tiles 2..15, qkT zone-B groups, pair-0 projection)
    drips independent PE work into the attention stream to keep the PE
    gapless (any engine-idle tick resets the p-state ramp).
  - output bf16 [T, C]; host accumulates the 4 tensor-parallel partials in
    fp32 and adds b_proj.
All matmul operands are bf16 (fp32 accumulate in PSUM).
"""

import os
import numpy as np
import ml_dtypes

import concourse.bacc as bacc
import concourse.mybir as mybir
import concourse.tile as tile
from concourse.bass_utils import run_bass_kernel_spmd
from concourse.masks import make_upper_triangular

B, T, C, H = 2, 2048, 1024, 16
D = C // H          # 64
HPC = H // 4        # 4 heads per core
QK = 2 * HPC * D    # 512 rows of qkT (q then k)
V = HPC * D         # 256 v columns
F32 = mybir.dt.float32
BF16 = mybir.dt.bfloat16
PAIR = 1024         # queries per attention pass (2 psum banks)
AF = mybir.ActivationFunctionType
NC_ = C // 128      # 8 c-tiles
TCH = 4             # 512-col xT chunks per c-tile
NWARM = 20          # warm-up matmuls covering the DMA lead-in

_cache = {}


def _sliced_dma(nc, dst, src, n=16):
    """DMA a [128, ...] tile as n partition-sliced calls (one per queue)."""
    step = 128 // n
    for k in range(n):
        nc.sync.dma_start(dst[k * step:(k + 1) * step],
                          src[k * step:(k + 1) * step])


def _build():
    nc = bacc.Bacc("TRN2", target_bir_lowering=False, debug=False, num_devices=8)
    # host-packed: xp[tch, p, c*512+t] = x[b, tch*512+t, c*128+p]
    xp = nc.dram_tensor("xp", [TCH, 128, NC_ * 512], BF16, kind="ExternalInput").ap()
    # wqkp[p, c*512+j] = w_qk[c*128+p, j]
    wqkp = nc.dram_tensor("wqkp", [128, NC_ * QK], BF16, kind="ExternalInput").ap()
    bqk_r = nc.dram_tensor("bqk_r", [1, QK], F32, kind="ExternalInput").ap()
    wvp = nc.dram_tensor("wvp", [128, NC_ * V], BF16, kind="ExternalInput").ap()
    bv_r = nc.dram_tensor("bv_r", [1, V], F32, kind="ExternalInput").ap()
    # wprp[p, k*1024+cc] = w_pr[k*128+p, cc]
    wprp = nc.dram_tensor("wprp", [128, (V // 128) * C], BF16, kind="ExternalInput").ap()
    out = nc.dram_tensor("out", [T, C], BF16, kind="ExternalOutput").ap()

    with tile.TileContext(nc) as tc:
        with (
            tc.tile_pool(name="const", bufs=1) as cpool,
            tc.tile_pool(name="xt", bufs=1) as xpool,
            tc.tile_pool(name="w", bufs=1) as wpool,
            tc.tile_pool(name="qk", bufs=1) as qkpool,
            tc.tile_pool(name="vaug", bufs=1) as vpool,
        ):
            # ---- tiny input DMAs first (single-descriptor rows) ----
            b_row = cpool.tile([1, QK], F32, name="b_row")
            nc.sync.dma_start(b_row[:], bqk_r[:])
            bv_row = cpool.tile([1, V], F32, name="bv_row")
            nc.sync.dma_start(bv_row[:], bv_r[:])

            # ---- big inputs, queue-sliced, in consumption order ----
            wqk_t = wpool.tile([128, NC_, QK], BF16, name="wqk")
            _sliced_dma(nc, wqk_t[:], wqkp.rearrange("p (c j) -> p c j", c=NC_))
            xtc = []
            for ch in range(2):
                t = xpool.tile([128, NC_, 512], BF16, name=f"xt{ch}")
                _sliced_dma(nc, t[:], xp[ch].rearrange("p (c j) -> p c j", c=NC_))
                xtc.append(t)
            wv_t = wpool.tile([128, NC_, V], BF16, name="wv")
            _sliced_dma(nc, wv_t[:], wvp.rearrange("p (c j) -> p c j", c=NC_), n=8)
            for ch in range(2, TCH):
                t = xpool.tile([128, NC_, 512], BF16, name=f"xt{ch}")
                _sliced_dma(nc, t[:], xp[ch].rearrange("p (c j) -> p c j", c=NC_))
                xtc.append(t)
            wpr_t = wpool.tile([128, V // 128, C], BF16, name="wpr")
            _sliced_dma(nc, wpr_t[:], wprp.rearrange("p (k j) -> p k j", k=V // 128),
                        n=8)

            # ---- constants / lead-in compute ----
            warm = cpool.tile([128, 512], BF16, name="warm")
            nc.gpsimd.memset(warm[:], 0.25)
            ident = cpool.tile([1, 1], F32, name="ident")
            nc.gpsimd.memset(ident[:], 1.0)
            # tri01[j, i] = 1 where i >= j: multiplicative causal mask for
            # the mixed diagonal 128x128 block (applied on DVE after exp)
            tri01 = cpool.tile([128, 128], BF16, name="tri01")
            make_upper_triangular(nc, tri01[:], val=1.0, diag=True)
            # preload the ACT exp table during the DMA lead-in
            warm_e = cpool.tile([1, 16], BF16, name="warm_e")
            nc.scalar.activation(warm_e[:], tri01[0:1, 0:16], AF.Exp)
            bv_full = cpool.tile([128, V], F32, name="bv_full")
            nc.gpsimd.partition_broadcast(bv_full[:], bv_row[:])

            # v tiles: whole tile preset to 1.0 (gpsimd, off critical path);
            # cols 64..127 of each head block overwritten with v rows later.
            v_t = [vpool.tile([128, HPC, 128], BF16, name=f"v{t}")
                   for t in range(T // 128)]
            for t in range(T // 128):
                nc.gpsimd.memset(v_t[t][:], 1.0)

            # q/k tiles split per 1024-col zone: zone B is written by filler
            # groups during pair-0 attention, so it must be a separate tile.
            qkZ = [[qkpool.tile([128, PAIR], BF16, name=f"qk{j}z{z}")
                    for z in range(2)] for j in range(QK // 128)]
            bqk_t = [cpool.tile([128, 1], F32, name=f"bqk{j}")
                     for j in range(QK // 128)]

            def make_v(tt, tag, bufs):
                """Produce v_t[tt] (8 accumulating matmuls + DVE bias add)."""
                def go(pool):
                    ps = pool.tile([128, V], F32, name="v_ps",
                                   tag=tag, bufs=bufs)
                    ch, sub = tt // 4, tt % 4
                    for c in range(NC_):
                        nc.tensor.matmul(
                            ps[:],
                            xtc[ch][:, c, sub * 128:(sub + 1) * 128],
                            wv_t[:, c, :],
                            start=(c == 0), stop=(c == NC_ - 1))
                    nc.vector.tensor_add(
                        v_t[tt][:, :, 64:64 + D],
                        ps[:].rearrange("p (h d) -> p h d", h=HPC),
                        bv_full[:].rearrange("p (h d) -> p h d", h=HPC))
                return go

            def qk_group(j, tch, pool, tag, bufs):
                ps = pool.tile([128, 512], F32, name="qk_ps",
                               tag=tag, bufs=bufs)
                for c in range(NC_):
                    nc.tensor.matmul(
                        ps[:],
                        wqk_t[:, c, j * 128:(j + 1) * 128],
                        xtc[tch][:, c, :],
                        start=(c == 0), stop=(c == NC_ - 1))
                nc.vector.tensor_scalar_add(
                    qkZ[j][tch // 2][:, (tch % 2) * 512:(tch % 2 + 1) * 512],
                    ps[:], bqk_t[j][:])

            with tc.tile_pool(name="ps1", bufs=1, space="PSUM") as ps1:
                # warm-up: keep the PE busy (and ramping) while DMA streams.
                # One never-consumed accumulation group on a ring slot; a
                # dummy DVE read frees the slot afterwards.
                wps = ps1.tile([128, 512], F32, name="warm_ps",
                               tag="qk_ps", bufs=8)
                for i in range(NWARM):
                    nc.tensor.matmul(wps[:], warm[:, 0:128], warm[:],
                                     start=(i == 0), stop=(i == NWARM - 1))
                wsink = cpool.tile([1, 4], F32, name="wsink")
                nc.vector.tensor_copy(wsink[:], wps[0:1, 0:4])

                # b_qk transposes: [1,128] row chunk -> [128,1] column
                for j in range(QK // 128):
                    pb = ps1.tile([128, 512], F32, name="b_ps",
                                  tag="qk_ps", bufs=8)
                    nc.tensor.transpose(pb[:, 0:1],
                                        b_row[:, j * 128:(j + 1) * 128],
                                        ident[:])
                    nc.vector.tensor_copy(bqk_t[j][:], pb[:, 0:1])

                # zone A only (queries/keys 0:1024); zone B groups become
                # pair-0 attention fillers
                for tch in range(2):
                    for j in range(QK // 128):
                        qk_group(j, tch, ps1, "qk_ps", 8)
                # first two v tiles before attention starts (share the ring)
                make_v(0, "qk_ps", 8)(ps1)
                make_v(1, "qk_ps", 8)(ps1)

            # ================= attention + projection =================
            with (
                tc.tile_pool(name="att_sb", bufs=1) as apool,
                tc.tile_pool(name="osb", bufs=1) as opool,
                tc.tile_pool(name="ps2", bufs=1, space="PSUM") as ps2,
            ):
                osb = {}
                yn_of = {}

                def proj_mm(p, tt, cc, copy_engine, tag, bufs):
                    """One projection psum group (2 matmuls + copy + DMA)."""
                    i0 = p * PAIR
                    def go(pool):
                        o_ps = pool.tile([128, 512], F32, name="o_ps",
                                         tag=tag, bufs=bufs)
                        for k in range(V // 128):
                            nc.tensor.matmul(
                                o_ps[:],
                                yn_of[p][k][:, tt * 128:(tt + 1) * 128],
                                wpr_t[:, k, cc * 512:(cc + 1) * 512],
                                start=(k == 0), stop=(k == V // 128 - 1))
                        if cc == 0:
                            osb[(p, tt)] = opool.tile(
                                [128, C], BF16, name="osb",
                                tag="osb", bufs=3)
                        ot = osb[(p, tt)]
                        copy_engine(ot[:, cc * 512:(cc + 1) * 512], o_ps[:])
                        if cc == C // 512 - 1:
                            for half in range(2):
                                nc.sync.dma_start(
                                    out[i0 + tt * 128 + half * 64:
                                        i0 + tt * 128 + (half + 1) * 64, :],
                                    ot[half * 64:(half + 1) * 64, :])
                    return go

                def qcol(j, lo, hi):
                    """Slice of q/k row-block j covering absolute cols
                    [lo:hi] (must stay inside one 1024-col zone)."""
                    z = lo // PAIR
                    return qkZ[j][z][:, lo - z * PAIR:hi - z * PAIR]

                # filler queue: independent PE work dripped into the
                # attention stream (keeps the PE gapless while ACT exps).
                # zone-B qkT groups first (pair-1 needs them), then v 8..11.
                fillers = [(lambda j=j, tch=tch:
                            (lambda pool: qk_group(j, tch, pool, "aux", 1)))()
                           for tch in (2, 3) for j in range(QK // 128)]
                fillers += [make_v(tt, "aux", 1) for tt in range(8, 12)]

                for p in (0, 1):
                    i0 = p * PAIR
                    njt = (i0 + PAIR) // 128
                    jlastA = (i0 + 512) // 128 - 1
                    yn = [apool.tile([128, PAIR], BF16, name=f"yn{k}",
                                     tag=f"yn{k}", bufs=2)
                          for k in range(V // 128)]
                    yn_of[p] = yn
                    for h in range(HPC):
                        qrow = (h % 2) * D
                        jq = h // 2
                        jk = 2 + h // 2
                        y_psA = ps2.tile([128, 512], F32, name="y_psA",
                                         tag="y_ps", bufs=3)
                        y_psB = ps2.tile([128, 512], F32, name="y_psB",
                                         tag="y_ps", bufs=3)
                        def issue_pv(jt, pTt):
                            dlt = max(0, jt * 128 - i0)
                            if dlt < 512:
                                nc.tensor.matmul(
                                    y_psA[:, dlt:512],
                                    v_t[jt][:, h, :],
                                    pTt[:, dlt:512],
                                    start=(jt == 0), stop=(jt == jlastA))
                            loB = max(512, dlt)
                            nc.tensor.matmul(
                                y_psB[:, loB - 512:512],
                                v_t[jt][:, h, :],
                                pTt[:, loB:PAIR],
                                start=(jt == 0), stop=(jt == njt - 1))

                        pv_q = []  # software-pipeline: PV issued 1 block late
                        for jt in range(njt):
                            j0 = jt * 128
                            dlt = max(0, j0 - i0)
                            s_ps = ps2.tile([128, PAIR], F32, name="s_ps",
                                            tag="s_ps", bufs=2)
                            pT = apool.tile([128, PAIR], BF16, name="pT",
                                            tag="pT", bufs=4)
                            diag = j0 >= i0
                            for sub in range(2):
                                lo = max(0, dlt - sub * 512)
                                if lo >= 512:
                                    continue
                                g0 = i0 + sub * 512
                                nc.tensor.matmul(
                                    s_ps[:, sub * 512 + lo:(sub + 1) * 512],
                                    qcol(jk, j0, j0 + 128)[qrow:qrow + D, :],
                                    qcol(jq, g0 + lo, g0 + 512)[qrow:qrow + D, :],
                                    start=True, stop=True)
                            nc.scalar.activation(
                                pT[:, dlt:PAIR], s_ps[:, dlt:PAIR], AF.Exp,
                                scale=float(1.0 / np.sqrt(D)))
                            if diag:
                                # zero the invalid (key > query) half of the
                                # mixed diagonal block on DVE
                                nc.vector.tensor_mul(
                                    pT[:, dlt:dlt + 128],
                                    pT[:, dlt:dlt + 128], tri01[:])
                            pv_q.append((jt, pT))
                            if len(pv_q) > 1:
                                issue_pv(*pv_q.pop(0))
                            # interleave one filler unit into the PE stream
                            if p == 0 and h == 0:
                                if jt < 6:
                                    make_v(jt + 2, "aux", 1)(ps2)
                            elif fillers and (
                                (p == 0 and jt % 2 == 0)
                                or (p == 1 and jt % 3 == 0)
                            ):
                                fillers.pop(0)(ps2)
                        while pv_q:
                            issue_pv(*pv_q.pop(0))
                        # normalize: psum rows 0..63 all hold l (ones cols of
                        # v tile); reciprocal directly on [64, N]
                        rec = apool.tile([D, PAIR], F32, name="rec",
                                         tag="rec", bufs=2)
                        nc.vector.reciprocal_approx_fast(
                            rec[:, 0:512], y_psA[0:D, :])
                        nc.vector.reciprocal_approx_fast(
                            rec[:, 512:PAIR], y_psB[0:D, :])
                        nc.vector.tensor_mul(
                            yn[jq][qrow:qrow + D, 0:512],
                            y_psA[D:2 * D, :], rec[:, 0:512])
                        nc.vector.tensor_mul(
                            yn[jq][qrow:qrow + D, 512:PAIR],
                            y_psB[D:2 * D, :], rec[:, 512:PAIR])
                    if p == 0:
                        # v 12..15 + projection of pair 0 fill pair-1 blocks
                        fillers.extend(make_v(tt, "aux", 1)
                                       for tt in range(12, 16))
                        fillers.extend(
                            proj_mm(0, tt, cc, nc.vector.tensor_copy,
                                    "aux", 1)
                            for tt in range(PAIR // 128)
                            for cc in range(C // 512))
                # drain leftovers, then tail: projection of pair 1 with
                # copies split ACT/DVE and a 3-deep psum ring
                while fillers:
                    fillers.pop(0)(ps2)
                for tt in range(PAIR // 128):
                    for cc in range(C // 512):
                        eng = (nc.scalar.copy if (tt + cc) % 2 == 0
                               else nc.vector.tensor_copy)
                        proj_mm(1, tt, cc, eng, "y_ps", 3)(ps2)
    nc.compile()
    return nc


def _get_nc():
    if "nc" not in _cache:
        _cache["nc"] = _build()
    return _cache["nc"]


def kernel(x, W_attn, b_attn, W_proj, b_proj):
    x = np.asarray(x, dtype=np.float32)
    W_attn = np.asarray(W_attn, dtype=np.float32)
    b_attn = np.asarray(b_attn, dtype=np.float32)
    W_proj = np.asarray(W_proj, dtype=np.float32)
    b_proj = np.asarray(b_proj, dtype=np.float32)

    nc = _get_nc()
    in_maps = []
    for c in range(8):
        b, g = c // 4, c % 4
        # [1024, N] -> [128, NC_*N] partition-packed (row p = concat over
        # c-tiles of source row c*128+p)
        def pack(m):
            n = m.shape[1]
            return np.ascontiguousarray(
                m.reshape(NC_, 128, n).transpose(1, 0, 2).reshape(128, NC_ * n)
            ).astype(ml_dtypes.bfloat16)

        xT = x[b].T  # [C, T]
        xp = np.ascontiguousarray(
            xT.reshape(NC_, 128, TCH, 512).transpose(2, 1, 0, 3)
            .reshape(TCH, 128, NC_ * 512)).astype(ml_dtypes.bfloat16)
        w_qk = np.concatenate([W_attn[:, g * V:(g + 1) * V],
                               W_attn[:, C + g * V:C + (g + 1) * V]], axis=1)
        b_qk = np.concatenate([b_attn[g * V:(g + 1) * V],
                               b_attn[C + g * V:C + (g + 1) * V]])
        w_pr = W_proj[g * V:(g + 1) * V, :]  # [256, 1024]
        in_maps.append({
            "xp": xp,
            "wqkp": pack(w_qk),
            "bqk_r": np.ascontiguousarray(b_qk.reshape(1, QK)),
            "wvp": pack(W_attn[:, 2 * C + g * V:2 * C + (g + 1) * V]),
            "bv_r": np.ascontiguousarray(
                b_attn[2 * C + g * V:2 * C + (g + 1) * V].reshape(1, V)),
            "wprp": np.ascontiguousarray(
                w_pr.reshape(V // 128, 128, C).transpose(1, 0, 2)
                .reshape(128, (V // 128) * C)).astype(ml_dtypes.bfloat16),
        })

    trace = os.environ.get("KTRACE") == "1"
    res = run_bass_kernel_spmd(nc, in_maps, core_ids=list(range(8)),
                               trace=trace)
    _cache["last_exec_ns"] = res.exec_time_ns
    _cache["last_result"] = res

    out = np.zeros((B, T, C), dtype=np.float32)
    for c in range(8):
        out[c // 4] += np.asarray(res.results[c]["out"], dtype=np.float32)
    out += b_proj[None, None, :]
    return out


# revision 8
# speedup vs baseline: 1.3004x; 1.3004x over previous
"""Causal self-attention (B=2, T=2048, C=1024, H=16) on 8 trn2 NeuronCores.

Sharding: core c handles batch b=c//4 and head group g=c%4 (4 heads each).
Data parallel on B, tensor parallel on H; W_attn/W_proj sliced per head
group; host sums the 4 tensor-parallel partial projection outputs per batch.

v3 layout notes (v2 spent 24us on the DMA lead-in -- each dma_start costs
~0.6us serially on the sync sequencer at ~116 descriptors/call, and
b_qk-as-[128,1] alone was 512 4-byte descriptors -- and ran the first ~35us
of matmuls at mid p-state):
  - all big inputs are HOST-PACKED so each SBUF tile is a contiguous DRAM
    block of 128 rows x (multi-KB); every fetch is 16 dma_starts of
    [8 partitions, whole-row] slices -> 8 descriptors/call (~100ns issue),
    one call per DMA queue. First qkT matmul inputs land ~7us in.
  - b_qk comes as [1, 512] (one descriptor) and is transposed to 4x[128,1]
    on the PE (permutation-matmul) during the lead-in.
  - ~20 warm-up matmuls on a dummy tile run while DMA streams, so the HAM
    p-state ramp (full speed only after 3us of continuous PE execution)
    completes before real work starts.
  - qkT runs only column-chunks 0/1 (queries 0:1024) up front; chunks 2/3
    are issued as fillers inside the pair-0 attention stream. q/k tiles are
    split per 1024-column zone (qkA/qkB) so pair-0 readers never serialize
    against the in-flight zone-B writers.
  - attention per (pair, head): S^T per 128-key block from the exact causal
    column into [128,1024] psum; ACT exp (scale=1/8) -> bf16 pT; mixed
    diagonal block masked multiplicatively on DVE; PV issued one block late
    with v tiles whose per-head 128 columns = [64 ones | 64 v], so psum
    rows 0..63 replicate the softmax denominator (reciprocal_approx_fast
    directly on [64,N], no partition broadcast).
  - filler queue (v tiles 2..15, qkT zone-B groups, pair-0 projection)
    drips independent PE work into the attention stream to keep the PE
    gapless (any engine-idle tick resets the p-state ramp).
  - output bf16 [T, C]; host accumulates the 4 tensor-parallel partials in
    fp32 and adds b_proj.
All matmul operands are bf16 (fp32 accumulate in PSUM).
"""

import os
import numpy as np
import ml_dtypes

import concourse.bacc as bacc
import concourse.mybir as mybir
import concourse.tile as tile
from concourse.bass_utils import run_bass_kernel_spmd
from concourse.masks import make_upper_triangular

B, T, C, H = 2, 2048, 1024, 16
D = C // H          # 64
HPC = H // 4        # 4 heads per core
QK = 2 * HPC * D    # 512 rows of qkT (q then k)
V = HPC * D         # 256 v columns
F32 = mybir.dt.float32
BF16 = mybir.dt.bfloat16
PAIR = 1024         # queries per attention pass (2 psum banks)
AF = mybir.ActivationFunctionType
NC_ = C // 128      # 8 c-tiles
TCH = 4             # 512-col xT chunks per c-tile
NWARM = 20          # warm-up matmuls covering the DMA lead-in

_cache = {}


def _sliced_dma(nc, dst, src, n=1):
    """DMA a [128, ...] tile. Full-height calls already spread descriptors
    across the DMA queues (queue = f(partition)); partition-sliced calls
    would concentrate a whole tensor onto few queues."""
    step = 128 // n
    for k in range(n):
        nc.sync.dma_start(dst[k * step:(k + 1) * step],
                          src[k * step:(k + 1) * step])


def _build():
    nc = bacc.Bacc("TRN2", target_bir_lowering=False, debug=False, num_devices=8)
    # host-packed: xp[tch, p, c*512+t] = x[b, tch*512+t, c*128+p]
    xp = nc.dram_tensor("xp", [TCH, 128, NC_ * 512], BF16, kind="ExternalInput").ap()
    # wqkp[p, c*512+j] = w_qk[c*128+p, j]
    wqkp = nc.dram_tensor("wqkp", [128, NC_ * QK], BF16, kind="ExternalInput").ap()
    bqk_r = nc.dram_tensor("bqk_r", [1, QK], F32, kind="ExternalInput").ap()
    wvp = nc.dram_tensor("wvp", [128, NC_ * V], BF16, kind="ExternalInput").ap()
    bv_r = nc.dram_tensor("bv_r", [1, V], F32, kind="ExternalInput").ap()
    # wprp[p, k*1024+cc] = w_pr[k*128+p, cc]
    wprp = nc.dram_tensor("wprp", [128, (V // 128) * C], BF16, kind="ExternalInput").ap()
    out = nc.dram_tensor("out", [T, C], BF16, kind="ExternalOutput").ap()

    with tile.TileContext(nc) as tc:
        with (
            tc.tile_pool(name="const", bufs=1) as cpool,
            tc.tile_pool(name="xt", bufs=1) as xpool,
            tc.tile_pool(name="w", bufs=1) as wpool,
            tc.tile_pool(name="qk", bufs=1) as qkpool,
            tc.tile_pool(name="vaug", bufs=1) as vpool,
        ):
            # ---- tiny input DMAs first (single-descriptor rows) ----
            b_row = cpool.tile([1, QK], F32, name="b_row")
            nc.sync.dma_start(b_row[:], bqk_r[:])
            bv_row = cpool.tile([1, V], F32, name="bv_row")
            nc.sync.dma_start(bv_row[:], bv_r[:])

            # ---- big inputs, queue-sliced, in consumption order ----
            wqk_t = wpool.tile([128, NC_, QK], BF16, name="wqk")
            _sliced_dma(nc, wqk_t[:], wqkp.rearrange("p (c j) -> p c j", c=NC_))
            xtc = []
            for ch in range(2):
                t = xpool.tile([128, NC_, 512], BF16, name=f"xt{ch}")
                _sliced_dma(nc, t[:], xp[ch].rearrange("p (c j) -> p c j", c=NC_))
                xtc.append(t)
            wv_t = wpool.tile([128, NC_, V], BF16, name="wv")
            _sliced_dma(nc, wv_t[:], wvp.rearrange("p (c j) -> p c j", c=NC_))
            for ch in range(2, TCH):
                t = xpool.tile([128, NC_, 512], BF16, name=f"xt{ch}")
                _sliced_dma(nc, t[:], xp[ch].rearrange("p (c j) -> p c j", c=NC_))
                xtc.append(t)
            wpr_t = wpool.tile([128, V // 128, C], BF16, name="wpr")
            _sliced_dma(nc, wpr_t[:], wprp.rearrange("p (k j) -> p k j", k=V // 128))

            # ---- constants / lead-in compute ----
            warm = cpool.tile([128, 512], BF16, name="warm")
            nc.gpsimd.memset(warm[:], 0.25)
            ident = cpool.tile([1, 1], F32, name="ident")
            nc.gpsimd.memset(ident[:], 1.0)
            # tri01[j, i] = 1 where i >= j: multiplicative causal mask for
            # the mixed diagonal 128x128 block (applied on DVE after exp)
            tri01 = cpool.tile([128, 128], BF16, name="tri01")
            make_upper_triangular(nc, tri01[:], val=1.0, diag=True)
            # preload the ACT exp table during the DMA lead-in
            warm_e = cpool.tile([1, 16], BF16, name="warm_e")
            nc.scalar.activation(warm_e[:], tri01[0:1, 0:16], AF.Exp)
            bv_full = cpool.tile([128, V], F32, name="bv_full")
            nc.gpsimd.partition_broadcast(bv_full[:], bv_row[:])

            # v tiles: whole tile preset to 1.0 (gpsimd, off critical path);
            # cols 64..127 of each head block overwritten with v rows later.
            v_t = [vpool.tile([128, HPC, 128], BF16, name=f"v{t}")
                   for t in range(T // 128)]
            for t in range(T // 128):
                nc.gpsimd.memset(v_t[t][:], 1.0)

            # q/k tiles split per 1024-col zone: zone B is written by filler
            # groups during pair-0 attention, so it must be a separate tile.
            qkZ = [[qkpool.tile([128, PAIR], BF16, name=f"qk{j}z{z}")
                    for z in range(2)] for j in range(QK // 128)]
            bqk_t = [cpool.tile([128, 1], F32, name=f"bqk{j}")
                     for j in range(QK // 128)]

            def make_v(tt, tag, bufs):
                """Produce v_t[tt] (8 accumulating matmuls + DVE bias add)."""
                def go(pool):
                    ps = pool.tile([128, V], F32, name="v_ps",
                                   tag=tag, bufs=bufs)
                    ch, sub = tt // 4, tt % 4
                    for c in range(NC_):
                        nc.tensor.matmul(
                            ps[:],
                            xtc[ch][:, c, sub * 128:(sub + 1) * 128],
                            wv_t[:, c, :],
                            start=(c == 0), stop=(c == NC_ - 1))
                    nc.vector.tensor_add(
                        v_t[tt][:, :, 64:64 + D],
                        ps[:].rearrange("p (h d) -> p h d", h=HPC),
                        bv_full[:].rearrange("p (h d) -> p h d", h=HPC))
                return go

            def qk_group(j, tch, pool, tag, bufs):
                ps = pool.tile([128, 512], F32, name="qk_ps",
                               tag=tag, bufs=bufs)
                for c in range(NC_):
                    nc.tensor.matmul(
                        ps[:],
                        wqk_t[:, c, j * 128:(j + 1) * 128],
                        xtc[tch][:, c, :],
                        start=(c == 0), stop=(c == NC_ - 1))
                nc.vector.tensor_scalar_add(
                    qkZ[j][tch // 2][:, (tch % 2) * 512:(tch % 2 + 1) * 512],
                    ps[:], bqk_t[j][:])

            with tc.tile_pool(name="ps1", bufs=1, space="PSUM") as ps1:
                # warm-up: keep the PE busy (and ramping) while DMA streams.
                # One never-consumed accumulation group on a ring slot; a
                # dummy DVE read frees the slot afterwards.
                wps = ps1.tile([128, 512], F32, name="warm_ps",
                               tag="qk_ps", bufs=8)
                for i in range(NWARM):
                    nc.tensor.matmul(wps[:], warm[:, 0:128], warm[:],
                                     start=(i == 0), stop=(i == NWARM - 1))
                wsink = cpool.tile([1, 4], F32, name="wsink")
                nc.vector.tensor_copy(wsink[:], wps[0:1, 0:4])

                # b_qk transposes: [1,128] row chunk -> [128,1] column
                for j in range(QK // 128):
                    pb = ps1.tile([128, 512], F32, name="b_ps",
                                  tag="qk_ps", bufs=8)
                    nc.tensor.transpose(pb[:, 0:1],
                                        b_row[:, j * 128:(j + 1) * 128],
                                        ident[:])
                    nc.vector.tensor_copy(bqk_t[j][:], pb[:, 0:1])

                # zone A only (queries/keys 0:1024); zone B groups become
                # pair-0 attention fillers
                for tch in range(2):
                    for j in range(QK // 128):
                        qk_group(j, tch, ps1, "qk_ps", 8)
                # first two v tiles before attention starts (share the ring)
                make_v(0, "qk_ps", 8)(ps1)
                make_v(1, "qk_ps", 8)(ps1)

            # ================= attention + projection =================
            with (
                tc.tile_pool(name="att_sb", bufs=1) as apool,
                tc.tile_pool(name="osb", bufs=1) as opool,
                tc.tile_pool(name="ps2", bufs=1, space="PSUM") as ps2,
            ):
                osb = {}
                yn_of = {}

                def proj_mm(p, tt, cc, copy_engine, tag, bufs):
                    """One projection psum group (2 matmuls + copy + DMA)."""
                    i0 = p * PAIR
                    def go(pool):
                        o_ps = pool.tile([128, 512], F32, name="o_ps",
                                         tag=tag, bufs=bufs)
                        for k in range(V // 128):
                            nc.tensor.matmul(
                                o_ps[:],
                                yn_of[p][k][:, tt * 128:(tt + 1) * 128],
                                wpr_t[:, k, cc * 512:(cc + 1) * 512],
                                start=(k == 0), stop=(k == V // 128 - 1))
                        if cc == 0:
                            osb[(p, tt)] = opool.tile(
                                [128, C], BF16, name="osb",
                                tag="osb", bufs=3)
                        ot = osb[(p, tt)]
                        copy_engine(ot[:, cc * 512:(cc + 1) * 512], o_ps[:])
                        if cc == C // 512 - 1:
                            for half in range(2):
                                nc.sync.dma_start(
                                    out[i0 + tt * 128 + half * 64:
                                        i0 + tt * 128 + (half + 1) * 64, :],
                                    ot[half * 64:(half + 1) * 64, :])
                    return go

                def qcol(j, lo, hi):
                    """Slice of q/k row-block j covering absolute cols
                    [lo:hi] (must stay inside one 1024-col zone)."""
                    z = lo // PAIR
                    return qkZ[j][z][:, lo - z * PAIR:hi - z * PAIR]

                # filler queue: independent PE work dripped into the
                # attention stream (keeps the PE gapless while ACT exps).
                # zone-B qkT groups first (pair-1 needs them), then v 8..11.
                fillers = [(lambda j=j, tch=tch:
                            (lambda pool: qk_group(j, tch, pool, "aux", 1)))()
                           for tch in (2, 3) for j in range(QK // 128)]
                fillers += [make_v(tt, "aux", 1) for tt in range(8, 12)]

                for p in (0, 1):
                    i0 = p * PAIR
                    njt = (i0 + PAIR) // 128
                    jlastA = (i0 + 512) // 128 - 1
                    yn = [apool.tile([128, PAIR], BF16, name=f"yn{k}",
                                     tag=f"yn{k}", bufs=2)
                          for k in range(V // 128)]
                    yn_of[p] = yn
                    for h in range(HPC):
                        qrow = (h % 2) * D
                        jq = h // 2
                        jk = 2 + h // 2
                        y_psA = ps2.tile([128, 512], F32, name="y_psA",
                                         tag="y_ps", bufs=3)
                        y_psB = ps2.tile([128, 512], F32, name="y_psB",
                                         tag="y_ps", bufs=3)
                        def issue_pv(jt, pTt):
                            dlt = max(0, jt * 128 - i0)
                            if dlt < 512:
                                nc.tensor.matmul(
                                    y_psA[:, dlt:512],
                                    v_t[jt][:, h, :],
                                    pTt[:, dlt:512],
                                    start=(jt == 0), stop=(jt == jlastA))
                            loB = max(512, dlt)
                            nc.tensor.matmul(
                                y_psB[:, loB - 512:512],
                                v_t[jt][:, h, :],
                                pTt[:, loB:PAIR],
                                start=(jt == 0), stop=(jt == njt - 1))

                        pv_q = []  # software-pipeline: PV issued 1 block late
                        for jt in range(njt):
                            j0 = jt * 128
                            dlt = max(0, j0 - i0)
                            s_ps = ps2.tile([128, PAIR], F32, name="s_ps",
                                            tag="s_ps", bufs=2)
                            pT = apool.tile([128, PAIR], BF16, name="pT",
                                            tag="pT", bufs=4)
                            diag = j0 >= i0
                            for sub in range(2):
                                lo = max(0, dlt - sub * 512)
                                if lo >= 512:
                                    continue
                                g0 = i0 + sub * 512
                                nc.tensor.matmul(
                                    s_ps[:, sub * 512 + lo:(sub + 1) * 512],
                                    qcol(jk, j0, j0 + 128)[qrow:qrow + D, :],
                                    qcol(jq, g0 + lo, g0 + 512)[qrow:qrow + D, :],
                                    start=True, stop=True)
                            nc.scalar.activation(
                                pT[:, dlt:PAIR], s_ps[:, dlt:PAIR], AF.Exp,
                                scale=float(1.0 / np.sqrt(D)))
                            if diag:
                                # zero the invalid (key > query) half of the
                                # mixed diagonal block on DVE
                                nc.vector.tensor_mul(
                                    pT[:, dlt:dlt + 128],
                                    pT[:, dlt:dlt + 128], tri01[:])
                            pv_q.append((jt, pT))
                            if len(pv_q) > 1:
                                issue_pv(*pv_q.pop(0))
                            # interleave one filler unit into the PE stream
                            if p == 0 and h == 0:
                                if jt < 6:
                                    make_v(jt + 2, "aux", 1)(ps2)
                            elif fillers and (
                                (p == 0 and jt % 2 == 0)
                                or (p == 1 and jt % 3 == 0)
                            ):
                                fillers.pop(0)(ps2)
                        while pv_q:
                            issue_pv(*pv_q.pop(0))
                        # normalize: psum rows 0..63 all hold l (ones cols of
                        # v tile); reciprocal directly on [64, N]
                        rec = apool.tile([D, PAIR], F32, name="rec",
                                         tag="rec", bufs=2)
                        nc.vector.reciprocal_approx_fast(
                            rec[:, 0:512], y_psA[0:D, :])
                        nc.vector.reciprocal_approx_fast(
                            rec[:, 512:PAIR], y_psB[0:D, :])
                        nc.vector.tensor_mul(
                            yn[jq][qrow:qrow + D, 0:512],
                            y_psA[D:2 * D, :], rec[:, 0:512])
                        nc.vector.tensor_mul(
                            yn[jq][qrow:qrow + D, 512:PAIR],
                            y_psB[D:2 * D, :], rec[:, 512:PAIR])
                    if p == 0:
                        # v 12..15 + projection of pair 0 fill pair-1 blocks
                        fillers.extend(make_v(tt, "aux", 1)
                                       for tt in range(12, 16))
                        fillers.extend(
                            proj_mm(0, tt, cc, nc.vector.tensor_copy,
                                    "aux", 1)
                            for tt in range(PAIR // 128)
                            for cc in range(C // 512))
                # drain leftovers, then tail: projection of pair 1 with
                # copies split ACT/DVE and a 3-deep psum ring
                while fillers:
                    fillers.pop(0)(ps2)
                for tt in range(PAIR // 128):
                    for cc in range(C // 512):
                        eng = (nc.scalar.copy if (tt + cc) % 2 == 0
                               else nc.vector.tensor_copy)
                        proj_mm(1, tt, cc, eng, "y_ps", 3)(ps2)
    nc.compile()
    return nc


def _get_nc():
    if "nc" not in _cache:
        _cache["nc"] = _build()
    return _cache["nc"]


def kernel(x, W_attn, b_attn, W_proj, b_proj):
    x = np.asarray(x, dtype=np.float32)
    W_attn = np.asarray(W_attn, dtype=np.float32)
    b_attn = np.asarray(b_attn, dtype=np.float32)
    W_proj = np.asarray(W_proj, dtype=np.float32)
    b_proj = np.asarray(b_proj, dtype=np.float32)

    nc = _get_nc()
    in_maps = []
    for c in range(8):
        b, g = c // 4, c % 4
        # [1024, N] -> [128, NC_*N] partition-packed (row p = concat over
        # c-tiles of source row c*128+p)
        def pack(m):
            n = m.shape[1]
            return np.ascontiguousarray(
                m.reshape(NC_, 128, n).transpose(1, 0, 2).reshape(128, NC_ * n)
            ).astype(ml_dtypes.bfloat16)

        xT = x[b].T  # [C, T]
        xp = np.ascontiguousarray(
            xT.reshape(NC_, 128, TCH, 512).transpose(2, 1, 0, 3)
            .reshape(TCH, 128, NC_ * 512)).astype(ml_dtypes.bfloat16)
        w_qk = np.concatenate([W_attn[:, g * V:(g + 1) * V],
                               W_attn[:, C + g * V:C + (g + 1) * V]], axis=1)
        b_qk = np.concatenate([b_attn[g * V:(g + 1) * V],
                               b_attn[C + g * V:C + (g + 1) * V]])
        w_pr = W_proj[g * V:(g + 1) * V, :]  # [256, 1024]
        in_maps.append({
            "xp": xp,
            "wqkp": pack(w_qk),
            "bqk_r": np.ascontiguousarray(b_qk.reshape(1, QK)),
            "wvp": pack(W_attn[:, 2 * C + g * V:2 * C + (g + 1) * V]),
            "bv_r": np.ascontiguousarray(
                b_attn[2 * C + g * V:2 * C + (g + 1) * V].reshape(1, V)),
            "wprp": np.ascontiguousarray(
                w_pr.reshape(V // 128, 128, C).transpose(1, 0, 2)
                .reshape(128, (V // 128) * C)).astype(ml_dtypes.bfloat16),
        })

    trace = os.environ.get("KTRACE") == "1"
    res = run_bass_kernel_spmd(nc, in_maps, core_ids=list(range(8)),
                               trace=trace)
    _cache["last_exec_ns"] = res.exec_time_ns
    _cache["last_result"] = res

    out = np.zeros((B, T, C), dtype=np.float32)
    for c in range(8):
        out[c // 4] += np.asarray(res.results[c]["out"], dtype=np.float32)
    out += b_proj[None, None, :]
    return out


# revision 10
# speedup vs baseline: 1.3143x; 1.0106x over previous
"""Causal self-attention (B=2, T=2048, C=1024, H=16) on 8 trn2 NeuronCores.

Sharding: core c handles batch b=c//4 and head group g=c%4 (4 heads each).
Data parallel on B, tensor parallel on H; W_attn/W_proj sliced per head
group; host sums the 4 tensor-parallel partial projection outputs per batch.

v3 layout notes (v2 spent 24us on the DMA lead-in -- each dma_start costs
~0.6us serially on the sync sequencer at ~116 descriptors/call, and
b_qk-as-[128,1] alone was 512 4-byte descriptors -- and ran the first ~35us
of matmuls at mid p-state):
  - all big inputs are HOST-PACKED so each SBUF tile is a contiguous DRAM
    block of 128 rows x (multi-KB); every fetch is 16 dma_starts of
    [8 partitions, whole-row] slices -> 8 descriptors/call (~100ns issue),
    one call per DMA queue. First qkT matmul inputs land ~7us in.
  - b_qk comes as [1, 512] (one descriptor) and is transposed to 4x[128,1]
    on the PE (permutation-matmul) during the lead-in.
  - ~20 warm-up matmuls on a dummy tile run while DMA streams, so the HAM
    p-state ramp (full speed only after 3us of continuous PE execution)
    completes before real work starts.
  - qkT runs only column-chunks 0/1 (queries 0:1024) up front; chunks 2/3
    are issued as fillers inside the pair-0 attention stream. q/k tiles are
    split per 1024-column zone (qkA/qkB) so pair-0 readers never serialize
    against the in-flight zone-B writers.
  - attention per (pair, head): S^T per 128-key block from the exact causal
    column into [128,1024] psum; ACT exp (scale=1/8) -> bf16 pT; mixed
    diagonal block masked multiplicatively on DVE; PV issued one block late
    with v tiles whose per-head 128 columns = [64 ones | 64 v], so psum
    rows 0..63 replicate the softmax denominator (reciprocal_approx_fast
    directly on [64,N], no partition broadcast).
  - filler queue (v tiles 2..15, qkT zone-B groups, pair-0 projection)
    drips independent PE work into the attention stream to keep the PE
    gapless (any engine-idle tick resets the p-state ramp).
  - output bf16 [T, C]; host accumulates the 4 tensor-parallel partials in
    fp32 and adds b_proj.
All matmul operands are bf16 (fp32 accumulate in PSUM).
"""

import os
import numpy as np
import ml_dtypes

import concourse.bacc as bacc
import concourse.mybir as mybir
import concourse.tile as tile
from concourse.bass_utils import run_bass_kernel_spmd
from concourse.masks import make_upper_triangular

B, T, C, H = 2, 2048, 1024, 16
D = C // H          # 64
HPC = H // 4        # 4 heads per core
QK = 2 * HPC * D    # 512 rows of qkT (q then k)
V = HPC * D         # 256 v columns
F32 = mybir.dt.float32
BF16 = mybir.dt.bfloat16
PAIR = 1024         # queries per attention pass (2 psum banks)
AF = mybir.ActivationFunctionType
NC_ = C // 128      # 8 c-tiles
TCH = 4             # 512-col xT chunks per c-tile
NWARM = 20          # warm-up matmuls covering the DMA lead-in

_cache = {}


def _sliced_dma(nc, dst, src, n=1):
    """DMA a [128, ...] tile. Full-height calls already spread descriptors
    across the DMA queues (queue = f(partition)); partition-sliced calls
    would concentrate a whole tensor onto few queues."""
    step = 128 // n
    for k in range(n):
        nc.sync.dma_start(dst[k * step:(k + 1) * step],
                          src[k * step:(k + 1) * step])


def _build():
    nc = bacc.Bacc("TRN2", target_bir_lowering=False, debug=False, num_devices=8)
    # host-packed: xp[tch, p, c*512+t] = x[b, tch*512+t, c*128+p]
    xp = nc.dram_tensor("xp", [TCH, 128, NC_ * 512], BF16, kind="ExternalInput").ap()
    # wqkp[p, j*1024 + c*128 + jj] = w_qk[c*128+p, j*128+jj]  (j-major)
    wqkp = nc.dram_tensor("wqkp", [128, NC_ * QK], BF16, kind="ExternalInput").ap()
    bqk_r = nc.dram_tensor("bqk_r", [1, QK], F32, kind="ExternalInput").ap()
    wvp = nc.dram_tensor("wvp", [128, NC_ * V], BF16, kind="ExternalInput").ap()
    bv_r = nc.dram_tensor("bv_r", [1, V], F32, kind="ExternalInput").ap()
    # wprp[p, k*1024+cc] = w_pr[k*128+p, cc]
    wprp = nc.dram_tensor("wprp", [128, (V // 128) * C], BF16, kind="ExternalInput").ap()
    out = nc.dram_tensor("out", [T, C], BF16, kind="ExternalOutput").ap()

    with tile.TileContext(nc) as tc:
        with (
            tc.tile_pool(name="const", bufs=1) as cpool,
            tc.tile_pool(name="xt", bufs=1) as xpool,
            tc.tile_pool(name="w", bufs=1) as wpool,
            tc.tile_pool(name="qk", bufs=1) as qkpool,
            tc.tile_pool(name="vaug", bufs=1) as vpool,
        ):
            # ---- tiny input DMAs first (single-descriptor rows) ----
            b_row = cpool.tile([1, QK], F32, name="b_row")
            nc.sync.dma_start(b_row[:], bqk_r[:])
            bv_row = cpool.tile([1, V], F32, name="bv_row")
            nc.sync.dma_start(bv_row[:], bv_r[:])

            # ---- big inputs, queue-sliced, in consumption order ----
            wqk_t = wpool.tile([128, QK // 128, NC_, 128], BF16, name="wqk")
            wqkp_v = wqkp.rearrange("p (j c k) -> p j c k", j=QK // 128, c=NC_)
            nc.sync.dma_start(wqk_t[:, 0], wqkp_v[:, 0])
            xtc = []
            for ch in range(2):
                t = xpool.tile([128, NC_, 512], BF16, name=f"xt{ch}")
                _sliced_dma(nc, t[:], xp[ch].rearrange("p (c j) -> p c j", c=NC_))
                xtc.append(t)
                if ch == 0:
                    nc.sync.dma_start(wqk_t[:, 1:], wqkp_v[:, 1:])
            wv_t = wpool.tile([128, NC_, V], BF16, name="wv")
            _sliced_dma(nc, wv_t[:], wvp.rearrange("p (c j) -> p c j", c=NC_))
            for ch in range(2, TCH):
                t = xpool.tile([128, NC_, 512], BF16, name=f"xt{ch}")
                _sliced_dma(nc, t[:], xp[ch].rearrange("p (c j) -> p c j", c=NC_))
                xtc.append(t)
            wpr_t = wpool.tile([128, V // 128, C], BF16, name="wpr")
            _sliced_dma(nc, wpr_t[:], wprp.rearrange("p (k j) -> p k j", k=V // 128))

            # ---- constants / lead-in compute ----
            warm = cpool.tile([128, 512], BF16, name="warm")
            nc.gpsimd.memset(warm[:], 0.25)
            ident = cpool.tile([1, 1], F32, name="ident")
            nc.gpsimd.memset(ident[:], 1.0)
            # tri01[j, i] = 1 where i >= j: multiplicative causal mask for
            # the mixed diagonal 128x128 block (applied on DVE after exp)
            tri01 = cpool.tile([128, 128], BF16, name="tri01")
            make_upper_triangular(nc, tri01[:], val=1.0, diag=True)
            # preload the ACT exp table during the DMA lead-in
            warm_e = cpool.tile([1, 16], BF16, name="warm_e")
            nc.scalar.activation(warm_e[:], tri01[0:1, 0:16], AF.Exp)
            bv_full = cpool.tile([128, V], F32, name="bv_full")
            nc.gpsimd.partition_broadcast(bv_full[:], bv_row[:])

            # v tiles: whole tile preset to 1.0 (gpsimd, off critical path);
            # cols 64..127 of each head block overwritten with v rows later.
            v_t = [vpool.tile([128, HPC, 128], BF16, name=f"v{t}")
                   for t in range(T // 128)]
            for t in range(T // 128):
                nc.gpsimd.memset(v_t[t][:], 1.0)

            # q/k tiles split per 1024-col zone: zone B is written by filler
            # groups during pair-0 attention, so it must be a separate tile.
            qkZ = [[qkpool.tile([128, PAIR], BF16, name=f"qk{j}z{z}")
                    for z in range(2)] for j in range(QK // 128)]
            bqk_t = [cpool.tile([128, 1], F32, name=f"bqk{j}")
                     for j in range(QK // 128)]

            def make_v(tt, tag, bufs):
                """Produce v_t[tt] (8 accumulating matmuls + DVE bias add)."""
                def go(pool):
                    ps = pool.tile([128, V], F32, name="v_ps",
                                   tag=tag, bufs=bufs)
                    ch, sub = tt // 4, tt % 4
                    for c in range(NC_):
                        nc.tensor.matmul(
                            ps[:],
                            xtc[ch][:, c, sub * 128:(sub + 1) * 128],
                            wv_t[:, c, :],
                            start=(c == 0), stop=(c == NC_ - 1))
                    nc.vector.tensor_add(
                        v_t[tt][:, :, 64:64 + D],
                        ps[:].rearrange("p (h d) -> p h d", h=HPC),
                        bv_full[:].rearrange("p (h d) -> p h d", h=HPC))
                return go

            def qk_group(j, tch, pool, tag, bufs):
                ps = pool.tile([128, 512], F32, name="qk_ps",
                               tag=tag, bufs=bufs)
                for c in range(NC_):
                    nc.tensor.matmul(
                        ps[:],
                        wqk_t[:, j, c, :],
                        xtc[tch][:, c, :],
                        start=(c == 0), stop=(c == NC_ - 1))
                nc.vector.tensor_scalar_add(
                    qkZ[j][tch // 2][:, (tch % 2) * 512:(tch % 2 + 1) * 512],
                    ps[:], bqk_t[j][:])

            with tc.tile_pool(name="ps1", bufs=1, space="PSUM") as ps1:
                # warm-up: keep the PE busy (and ramping) while DMA streams.
                # One never-consumed accumulation group on a ring slot; a
                # dummy DVE read frees the slot afterwards.
                wps = ps1.tile([128, 512], F32, name="warm_ps",
                               tag="qk_ps", bufs=8)
                for i in range(NWARM):
                    nc.tensor.matmul(wps[:], warm[:, 0:128], warm[:],
                                     start=(i == 0), stop=(i == NWARM - 1))
                wsink = cpool.tile([1, 4], F32, name="wsink")
                nc.vector.tensor_copy(wsink[:], wps[0:1, 0:4])

                # b_qk transposes: [1,128] row chunk -> [128,1] column
                for j in range(QK // 128):
                    pb = ps1.tile([128, 512], F32, name="b_ps",
                                  tag="qk_ps", bufs=8)
                    nc.tensor.transpose(pb[:, 0:1],
                                        b_row[:, j * 128:(j + 1) * 128],
                                        ident[:])
                    nc.vector.tensor_copy(bqk_t[j][:], pb[:, 0:1])

                # zone A only (queries/keys 0:1024); zone B groups become
                # pair-0 attention fillers
                for tch in range(2):
                    for j in range(QK // 128):
                        qk_group(j, tch, ps1, "qk_ps", 8)
                # first two v tiles before attention starts (share the ring)
                make_v(0, "qk_ps", 8)(ps1)
                make_v(1, "qk_ps", 8)(ps1)

            # ================= attention + projection =================
            with (
                tc.tile_pool(name="att_sb", bufs=1) as apool,
                tc.tile_pool(name="osb", bufs=1) as opool,
                tc.tile_pool(name="ps2", bufs=1, space="PSUM") as ps2,
            ):
                osb = {}
                yn_of = {}

                def proj_mm(p, tt, cc, copy_engine, tag, bufs):
                    """One projection psum group (2 matmuls + copy + DMA)."""
                    i0 = p * PAIR
                    def go(pool):
                        o_ps = pool.tile([128, 512], F32, name="o_ps",
                                         tag=tag, bufs=bufs)
                        for k in range(V // 128):
                            nc.tensor.matmul(
                                o_ps[:],
                                yn_of[p][k][:, tt * 128:(tt + 1) * 128],
                                wpr_t[:, k, cc * 512:(cc + 1) * 512],
                                start=(k == 0), stop=(k == V // 128 - 1))
                        if cc == 0:
                            osb[(p, tt)] = opool.tile(
                                [128, C], BF16, name="osb",
                                tag="osb", bufs=3)
                        ot = osb[(p, tt)]
                        copy_engine(ot[:, cc * 512:(cc + 1) * 512], o_ps[:])
                        if cc == C // 512 - 1:
                            for half in range(2):
                                nc.sync.dma_start(
                                    out[i0 + tt * 128 + half * 64:
                                        i0 + tt * 128 + (half + 1) * 64, :],
                                    ot[half * 64:(half + 1) * 64, :])
                    return go

                def qcol(j, lo, hi):
                    """Slice of q/k row-block j covering absolute cols
                    [lo:hi] (must stay inside one 1024-col zone)."""
                    z = lo // PAIR
                    return qkZ[j][z][:, lo - z * PAIR:hi - z * PAIR]

                # filler queue: independent PE work dripped into the
                # attention stream (keeps the PE gapless while ACT exps).
                # zone-B qkT groups first (pair-1 needs them), then v 8..11.
                fillers = [(lambda j=j, tch=tch:
                            (lambda pool: qk_group(j, tch, pool, "aux", 1)))()
                           for tch in (2, 3) for j in range(QK // 128)]
                fillers += [make_v(tt, "aux", 1) for tt in range(8, 12)]

                for p in (0, 1):
                    i0 = p * PAIR
                    njt = (i0 + PAIR) // 128
                    jlastA = (i0 + 512) // 128 - 1
                    yn = [apool.tile([128, PAIR], BF16, name=f"yn{k}",
                                     tag=f"yn{k}", bufs=2)
                          for k in range(V // 128)]
                    yn_of[p] = yn
                    for h in range(HPC):
                        qrow = (h % 2) * D
                        jq = h // 2
                        jk = 2 + h // 2
                        y_psA = ps2.tile([128, 512], F32, name="y_psA",
                                         tag="y_ps", bufs=3)
                        y_psB = ps2.tile([128, 512], F32, name="y_psB",
                                         tag="y_ps", bufs=3)
                        def issue_pv(jt, pTt):
                            dlt = max(0, jt * 128 - i0)
                            if dlt < 512:
                                nc.tensor.matmul(
                                    y_psA[:, dlt:512],
                                    v_t[jt][:, h, :],
                                    pTt[:, dlt:512],
                                    start=(jt == 0), stop=(jt == jlastA))
                            loB = max(512, dlt)
                            nc.tensor.matmul(
                                y_psB[:, loB - 512:512],
                                v_t[jt][:, h, :],
                                pTt[:, loB:PAIR],
                                start=(jt == 0), stop=(jt == njt - 1))

                        pv_q = []  # software-pipeline: PV issued 1 block late
                        for jt in range(njt):
                            j0 = jt * 128
                            dlt = max(0, j0 - i0)
                            s_ps = ps2.tile([128, PAIR], F32, name="s_ps",
                                            tag="s_ps", bufs=2)
                            pT = apool.tile([128, PAIR], BF16, name="pT",
                                            tag="pT", bufs=6)
                            diag = j0 >= i0
                            for sub in range(2):
                                lo = max(0, dlt - sub * 512)
                                if lo >= 512:
                                    continue
                                g0 = i0 + sub * 512
                                nc.tensor.matmul(
                                    s_ps[:, sub * 512 + lo:(sub + 1) * 512],
                                    qcol(jk, j0, j0 + 128)[qrow:qrow + D, :],
                                    qcol(jq, g0 + lo, g0 + 512)[qrow:qrow + D, :],
                                    start=True, stop=True)
                            nc.scalar.activation(
                                pT[:, dlt:PAIR], s_ps[:, dlt:PAIR], AF.Exp,
                                scale=float(1.0 / np.sqrt(D)))
                            if diag:
                                # zero the invalid (key > query) half of the
                                # mixed diagonal block on DVE
                                nc.vector.tensor_mul(
                                    pT[:, dlt:dlt + 128],
                                    pT[:, dlt:dlt + 128], tri01[:])
                            pv_q.append((jt, pT))
                            if len(pv_q) > 2:
                                issue_pv(*pv_q.pop(0))
                            # interleave one filler unit into the PE stream
                            if p == 0 and h == 0:
                                if jt < 6:
                                    make_v(jt + 2, "aux", 1)(ps2)
                            elif fillers and (
                                (p == 0 and jt % 2 == 0)
                                or (p == 1 and jt % 2 == 0)
                            ):
                                fillers.pop(0)(ps2)
                        while pv_q:
                            issue_pv(*pv_q.pop(0))
                        # normalize: psum rows 0..63 all hold l (ones cols of
                        # v tile); reciprocal directly on [64, N]
                        rec = apool.tile([D, PAIR], F32, name="rec",
                                         tag="rec", bufs=2)
                        nc.vector.reciprocal_approx_fast(
                            rec[:, 0:512], y_psA[0:D, :])
                        nc.vector.reciprocal_approx_fast(
                            rec[:, 512:PAIR], y_psB[0:D, :])
                        nc.vector.tensor_mul(
                            yn[jq][qrow:qrow + D, 0:512],
                            y_psA[D:2 * D, :], rec[:, 0:512])
                        nc.vector.tensor_mul(
                            yn[jq][qrow:qrow + D, 512:PAIR],
                            y_psB[D:2 * D, :], rec[:, 512:PAIR])
                    if p == 0:
                        # v 12..15 + projection of pair 0 fill pair-1 blocks
                        fillers.extend(make_v(tt, "aux", 1)
                                       for tt in range(12, 16))
                        fillers.extend(
                            proj_mm(0, tt, cc, nc.vector.tensor_copy,
                                    "aux", 1)
                            for tt in range(PAIR // 128)
                            for cc in range(C // 512))
                # drain leftovers, then tail: projection of pair 1 with
                # copies split ACT/DVE and a 3-deep psum ring
                while fillers:
                    fillers.pop(0)(ps2)
                for tt in range(PAIR // 128):
                    for cc in range(C // 512):
                        eng = (nc.scalar.copy if (tt + cc) % 2 == 0
                               else nc.vector.tensor_copy)
                        proj_mm(1, tt, cc, eng, "y_ps", 3)(ps2)
    nc.compile()
    return nc


def _get_nc():
    if "nc" not in _cache:
        _cache["nc"] = _build()
    return _cache["nc"]


def kernel(x, W_attn, b_attn, W_proj, b_proj):
    x = np.asarray(x, dtype=np.float32)
    W_attn = np.asarray(W_attn, dtype=np.float32)
    b_attn = np.asarray(b_attn, dtype=np.float32)
    W_proj = np.asarray(W_proj, dtype=np.float32)
    b_proj = np.asarray(b_proj, dtype=np.float32)

    nc = _get_nc()
    in_maps = []
    for c in range(8):
        b, g = c // 4, c % 4
        # [1024, N] -> [128, NC_*N] partition-packed (row p = concat over
        # c-tiles of source row c*128+p)
        def pack(m):
            n = m.shape[1]
            return np.ascontiguousarray(
                m.reshape(NC_, 128, n).transpose(1, 0, 2).reshape(128, NC_ * n)
            ).astype(ml_dtypes.bfloat16)

        xT = x[b].T  # [C, T]
        xp = np.ascontiguousarray(
            xT.reshape(NC_, 128, TCH, 512).transpose(2, 1, 0, 3)
            .reshape(TCH, 128, NC_ * 512)).astype(ml_dtypes.bfloat16)
        w_qk = np.concatenate([W_attn[:, g * V:(g + 1) * V],
                               W_attn[:, C + g * V:C + (g + 1) * V]], axis=1)
        b_qk = np.concatenate([b_attn[g * V:(g + 1) * V],
                               b_attn[C + g * V:C + (g + 1) * V]])
        w_pr = W_proj[g * V:(g + 1) * V, :]  # [256, 1024]
        wqkp = np.ascontiguousarray(
            w_qk.reshape(NC_, 128, QK // 128, 128).transpose(1, 2, 0, 3)
            .reshape(128, NC_ * QK)).astype(ml_dtypes.bfloat16)
        in_maps.append({
            "xp": xp,
            "wqkp": wqkp,
            "bqk_r": np.ascontiguousarray(b_qk.reshape(1, QK)),
            "wvp": pack(W_attn[:, 2 * C + g * V:2 * C + (g + 1) * V]),
            "bv_r": np.ascontiguousarray(
                b_attn[2 * C + g * V:2 * C + (g + 1) * V].reshape(1, V)),
            "wprp": np.ascontiguousarray(
                w_pr.reshape(V // 128, 128, C).transpose(1, 0, 2)
                .reshape(128, (V // 128) * C)).astype(ml_dtypes.bfloat16),
        })

    trace = os.environ.get("KTRACE") == "1"
    res = run_bass_kernel_spmd(nc, in_maps, core_ids=list(range(8)),
                               trace=trace)
    _cache["last_exec_ns"] = res.exec_time_ns
    _cache["last_result"] = res

    out = np.zeros((B, T, C), dtype=np.float32)
    for c in range(8):
        out[c // 4] += np.asarray(res.results[c]["out"], dtype=np.float32)
    out += b_proj[None, None, :]
    return out
